# revision 12
# baseline (speedup 1.0000x reference)
"""IoU metric kernel for Trainium2 (Bass/Tile), 8-core data-parallel over batch.

Problem: input [16,21,512,512] f32 logits, target [16,21,512,512] f32 0/1 masks.
  pred = argmax_C(input); per-(b,c): inter = sum(target * onehot(pred)),
  gt = sum(target), pr = sum(onehot(pred)); present = any(target) = (gt > 0).
  scores[c] = (sum_b present*inter) / (sum_b present*(gt+pr) - inter_s + eps) * counts
Returns (scores[1:], counts[1:]).

Sharding: batch 16 -> 8 cores x 2 images. Each core computes per-image [C,3]
partials (inter, gt, pr); host does the trivial cross-batch combine.

Per-core kernel layout: image pixel plane [512,512] split into chunks of 128
h-rows: tile [128 part, 21 classes, 512 w]. Engines:
  DVE : 20-op running-max chain + 21 fused tensor_tensor_reduce(is_equal) ops
        producing the one-hot (bf16) and chained per-class pr accumulators.
  POOL: one big oh *= t multiply (bf16, in-place) - offloads DVE.
  PE  : per-class ones-matmuls accumulating inter/gt into PSUM across chunks.
  DMA : HWDGE (sync) for x; SWDGE (gpsimd) f32->bf16 cast for target.
"""

import os
import threading
from contextlib import ExitStack

import numpy as np

import concourse.bacc as bacc
import concourse.bass as bass
import concourse.mybir as mybir
import concourse.tile as tile
from concourse.alu_op_type import AluOpType
from concourse.bass_utils import run_bass_kernel_spmd

F32 = mybir.dt.float32
BF16 = mybir.dt.bfloat16

B, C, H, W = 16, 21, 512, 512
NCORES = 8
BPC = B // NCORES  # images per core
P = 128

# Tunables
USE_POOL_MUL = os.environ.get("IOU_POOL_MUL", "1") == "1"
T_BF16 = os.environ.get("IOU_T_BF16", "1") == "1"
INPLACE_MUL = os.environ.get("IOU_INPLACE_MUL", "1") == "1"


def build_kernel_ir(nc, bpc=BPC, n_classes=C, h=H, w=W):
    """Emit the Tile IR for one core's shard [bpc, n_classes, h, w]."""
    f = w  # free dim per chunk = image width
    chunks = h // P  # chunks per image (h rows of 128)

    inp = nc.dram_tensor("input", [bpc, n_classes, h, w], F32, kind="ExternalInput")
    tgt = nc.dram_tensor("target", [bpc, n_classes, h, w], F32, kind="ExternalInput")
    stats = nc.dram_tensor("stats", [bpc, n_classes, 4], F32, kind="ExternalOutput")

    # [b, c, (j p), w] -> [b, j, p, c, w]
    inp_r = inp.ap().rearrange("b c (j p) w -> b j p c w", p=P)
    tgt_r = tgt.ap().rearrange("b c (j p) w -> b j p c w", p=P)
    stats_ap = stats.ap()

    t_dt = BF16 if T_BF16 else F32

    with tile.TileContext(nc) as tc, ExitStack() as ctx:
        data_pool = ctx.enter_context(tc.tile_pool(name="data", bufs=2))
        acc_pool = ctx.enter_context(tc.tile_pool(name="acc", bufs=1))
        out_pool = ctx.enter_context(tc.tile_pool(name="outp", bufs=1))
        psum_pool = ctx.enter_context(tc.tile_pool(name="psum", bufs=1, space="PSUM"))

        # Per-class selector weights: E[:, c, :] is [128, C] with column c all
        # ones -> matmul(E_c.T @ rhs) adds colsum(rhs) into PSUM row c only.
        sel_dt = BF16 if T_BF16 else F32
        sel = acc_pool.tile([P, n_classes, n_classes], sel_dt, tag="sel")
        nc.vector.memset(sel, 0.0)
        for c in range(n_classes):
            nc.vector.memset(sel[:, c, c : c + 1], 1.0)

        for img in range(bpc):
            psum_inter = psum_pool.tile([n_classes, f], F32, tag=f"pi{img}")
            psum_gt = psum_pool.tile([n_classes, f], F32, tag=f"pg{img}")
            psum_pr = psum_pool.tile([n_classes, f], F32, tag=f"pp{img}")

            for j in range(chunks):
                xb = data_pool.tile([P, n_classes, f], F32, tag="xb")
                nc.sync.dma_start(out=xb[:], in_=inp_r[img, j])
                tb = data_pool.tile([P, n_classes, f], t_dt, tag="tb")
                if T_BF16:
                    nc.gpsimd.dma_start(out=tb[:], in_=tgt_r[img, j])
                else:
                    nc.sync.dma_start(out=tb[:], in_=tgt_r[img, j])

                # running max over classes
                m = data_pool.tile([P, f], F32, tag="m")
                nc.vector.tensor_max(m[:], xb[:, 0, :], xb[:, 1, :])
                for c in range(2, n_classes):
                    nc.vector.tensor_max(m[:], m[:], xb[:, c, :])

                # one-hot via is_equal vs the max
                oh_dt = BF16 if T_BF16 else F32
                oh = data_pool.tile(
                    [P, n_classes, f], oh_dt, tag="oh",
                    bufs=(2 if INPLACE_MUL else 1),
                )
                for c in range(n_classes):
                    nc.vector.tensor_tensor(
                        oh[:, c, :], xb[:, c, :], m[:], AluOpType.is_equal
                    )

                # pr matmuls must read oh before the (possibly in-place) mul
                for c in range(n_classes):
                    first = j == 0 and c == 0
                    last = j == chunks - 1 and c == n_classes - 1
                    nc.tensor.matmul(
                        psum_pr[:, :], sel[:, c, :], oh[:, c, :],
                        start=first, stop=last,
                    )
                    nc.tensor.matmul(
                        psum_gt[:, :], sel[:, c, :], tb[:, c, :],
                        start=first, stop=last,
                    )

                # prod = oh * t
                if INPLACE_MUL:
                    prod = oh
                else:
                    prod = data_pool.tile([P, n_classes, f], oh_dt, tag="prod", bufs=1)
                if USE_POOL_MUL:
                    nc.gpsimd.tensor_mul(prod[:], oh[:], tb[:])
                else:
                    nc.vector.tensor_mul(prod[:], oh[:], tb[:])

                for c in range(n_classes):
                    first = j == 0 and c == 0
                    last = j == chunks - 1 and c == n_classes - 1
                    nc.tensor.matmul(
                        psum_inter[:, :], sel[:, c, :], prod[:, c, :],
                        start=first, stop=last,
                    )

            # finalize image: [C,f] psum -> [C,1]; pr partition-reduce via PE
            res = out_pool.tile([n_classes, 4], F32, tag=f"res{img}")
            nc.vector.tensor_reduce(
                out=res[:, 0:1], in_=psum_inter[:], axis=mybir.AxisListType.X,
                op=AluOpType.add,
            )
            nc.vector.tensor_reduce(
                out=res[:, 1:2], in_=psum_gt[:], axis=mybir.AxisListType.X,
                op=AluOpType.add,
            )
            nc.vector.tensor_reduce(
                out=res[:, 2:3], in_=psum_pr[:], axis=mybir.AxisListType.X,
                op=AluOpType.add,
            )
            nc.vector.memset(res[:, 3:4], 0.0)
            nc.sync.dma_start(out=stats_ap[img], in_=res[:])

    return nc


_BUILD_LOCK = threading.Lock()
_NC_CACHE = {}


def get_compiled_nc(key="full"):
    with _BUILD_LOCK:
        if key not in _NC_CACHE:
            nc = bacc.Bacc("TRN2", target_bir_lowering=False, debug=False)
            build_kernel_ir(nc)
            nc.compile()
            _NC_CACHE[key] = nc
        return _NC_CACHE[key]


def combine_stats(stats_all):
    """stats_all: [B, C, >=3] per-image partials -> (scores[1:], counts[1:])."""
    stats_all = np.asarray(stats_all, dtype=np.float64)
    inter_bc = stats_all[..., 0]
    gt_bc = stats_all[..., 1]
    pr_bc = stats_all[..., 2]
    present = (gt_bc > 0).astype(np.float64)
    inter_s = (present * inter_bc).sum(0)
    union_s = (present * (gt_bc + pr_bc)).sum(0) - inter_s + 1e-7
    counts = present.sum(0)
    scores = (inter_s / union_s) * counts
    return (
        scores[1:].astype(np.float32),
        counts[1:].astype(np.float32),
    )


def kernel(input, target):
    inp = np.ascontiguousarray(np.asarray(input, dtype=np.float32))
    tgt = np.ascontiguousarray(np.asarray(target, dtype=np.float32))
    assert inp.shape == (B, C, H, W), inp.shape

    nc = get_compiled_nc()
    in_maps = [
        {
            "input": inp[i * BPC : (i + 1) * BPC],
            "target": tgt[i * BPC : (i + 1) * BPC],
        }
        for i in range(NCORES)
    ]
    res = run_bass_kernel_spmd(nc, in_maps, core_ids=list(range(NCORES)))
    stats_all = np.concatenate([r["stats"] for r in res.results], axis=0)  # [B,C,4]
    return combine_stats(stats_all)


if __name__ == "__main__":
    rng = np.random.default_rng(0)
    x = rng.standard_normal((B, C, H, W), dtype=np.float32)
    t = (rng.random((B, C, H, W)) < 0.05).astype(np.float32)
    s, c = kernel(input=x, target=t)
    print("scores:", s)
    print("counts:", c)
